# revision 1
# baseline (speedup 1.0000x reference)
"""Trainium2 Bass kernel for nn_ConvBundle_48146583388363.

Math: out[x,y,b,i,j,o] = s[b, i+x-1, j+y-1] * wsum[x,y,o]
  where s = inputs.sum(channel) (zero-padded at borders) and
  wsum = W.sum(axis=2).

Sharding: data-parallel over batch B=16 across 8 cores (2 batches/core).
W and the small structural constants are replicated.

Per-core layout: flattened per-batch spatial index f = 128*t + p
(p = SBUF partition, t = tile column). The 9 tap shifts f -> f+delta
are done with 0/1 shift-matrix matmuls on the tensor engine (plus a
column-border mask), then each output tile [128 spatial, 128 cout] is a
per-partition tensor_scalar outer product, accumulated into [128, 9216]
slabs and DMA'd out as one multi-MB transfer per (tap, batch).

Note: walrus allows only ONE sync-wait on a Matmult (it rides the
LDWEIGHTS struct), so matmul operands are grouped into single DMAs and
a dummy matmul pre-syncs the shift-matrix DMA lane on PE.
"""

import numpy as np

import concourse.bacc as bacc
import concourse.bass as bass
import concourse.mybir as mybir
from concourse import tile
from concourse.bass_utils import run_bass_kernel_spmd

F32 = mybir.dt.float32

NCORES = 8
B, H, W_, CIN = 16, 96, 96, 64
COUT = 128
BPC = B // NCORES          # batches per core = 2
SP = H * W_                # 9216 spatial positions per batch
TPB = SP // 128            # 72 tiles of 128 spatial positions
NTAP = 9
TAPS = [(x - 1, y - 1) for x in range(3) for y in range(3)]  # tap n = 3x+y


def _build_consts():
    """Structural (input-independent) constants, computed on host."""
    shift_ab = np.zeros((2 * NTAP, 128, 128), np.float32)
    for n, (dx, dy) in enumerate(TAPS):
        d = 96 * dx + dy
        if d == 0:
            continue
        for m in range(128):
            k = m + d
            if 0 <= k < 128:
                shift_ab[n, k, m] = 1.0
            elif d > 0:
                shift_ab[NTAP + n, k - 128, m] = 1.0
            else:
                shift_ab[NTAP + n, k + 128, m] = 1.0
    f = 128 * np.arange(TPB)[None, :] + np.arange(128)[:, None]  # [128, 72]
    masks = np.stack([f % 96 != 0, f % 96 != 95]).astype(np.float32)
    return shift_ab, masks


def _build_nc():
    # Bacc (not raw Bass): its finalize() runs move_matmul_waits_to_ldweights
    # + generate_event_semaphores, which split multi-waits to satisfy the
    # 1-sync-wait-per-instruction hardware constraint.
    nc = bacc.Bacc(None, target_bir_lowering=False)
    x = nc.dram_tensor("x", [BPC, SP, CIN], F32, kind="ExternalInput")
    # wc[0] = all-ones (for the colsum matmul), wc[1+n] = W tap n
    wc = nc.dram_tensor("wc", [1 + NTAP, 128, COUT], F32, kind="ExternalInput")
    ab = nc.dram_tensor("ab", [2 * NTAP, 128, 128], F32, kind="ExternalInput")
    mk = nc.dram_tensor("mk", [2, 128, TPB], F32, kind="ExternalInput")
    # y is stored (p, t, o) per (tap, batch): partition-major, so each
    # partition's 72*128 floats are one contiguous 36.9KB DRAM run and the
    # slab DMA is fully linear. Host unshard permutes (p,t)->(t,p).
    y = nc.dram_tensor("y", [NTAP, BPC, 128, TPB * COUT], F32, kind="ExternalOutput")

    with tile.TileContext(nc) as tc:
        with (
            tc.tile_pool(name="const", bufs=1) as cpool,
            tc.tile_pool(name="xin", bufs=2) as xpool,
            tc.tile_pool(name="sshift", bufs=4) as spool,
            tc.tile_pool(name="psum_w", bufs=2, space="PSUM") as pwpool,
            tc.tile_pool(name="psum_s", bufs=4, space="PSUM") as pspool,
            tc.tile_pool(name="out", bufs=6) as opool,
        ):
            # Batch loads first on the ACT HWDGE ring (critical path to the
            # first slab); consts go on the otherwise-idle SP ring. Loads are
            # chunked in t-quarters so the first reduce (and the center tap's
            # output stream) starts after ~1/4 of the load.
            NQ = 2
            qt = TPB // NQ
            # Batch 0's two halves land in parallel on both HWDGE rings (it
            # gates the first slabs); consts follow on the sync ring; batch 1
            # streams on the scalar ring.
            xts = []
            for b in range(BPC):
                xt = xpool.tile([128, TPB * CIN], F32, name=f"xt{b}", tag="xt")
                xts.append(xt)

            def _load_x(b, q, eng):
                xsrc = x[b].rearrange("(t p) c -> p t c", p=128)
                eng.dma_start(
                    out=xts[b][:, q * qt * CIN:(q + 1) * qt * CIN],
                    in_=xsrc[:, q * qt:(q + 1) * qt],
                )

            _load_x(0, 0, nc.scalar)
            _load_x(0, 1, nc.sync)

            wc_sb = cpool.tile([128, (1 + NTAP) * COUT], F32, name="wc_sb")
            nc.sync.dma_start(out=wc_sb[:], in_=wc.rearrange("n k m -> k n m"))
            ab_sb = cpool.tile([128, 2 * NTAP * 128], F32, name="ab_sb")
            nc.sync.dma_start(out=ab_sb[:], in_=ab.rearrange("n k m -> k n m"))
            mk_sb = cpool.tile([128, 2 * TPB], F32, name="mk_sb")
            nc.sync.dma_start(out=mk_sb[:], in_=mk.rearrange("n p t -> p n t"))

            _load_x(1, 0, nc.scalar)
            _load_x(1, 1, nc.scalar)

            # wsum[n] = colsum of W[n], replicated across all 128 partitions
            # via ones.T @ W (one matmul does reduce + broadcast).
            ones_ap = wc_sb[:, 0:COUT]
            wsum = []
            for n in range(NTAP):
                pw = pwpool.tile([128, COUT], F32, name=f"pw{n}", tag="pw")
                nc.tensor.matmul(
                    pw[:], lhsT=ones_ap,
                    rhs=wc_sb[:, (1 + n) * COUT:(2 + n) * COUT],
                    start=True, stop=True,
                )
                ws = cpool.tile([128, COUT], F32, name=f"wsum{n}")
                nc.scalar.copy(ws[:], pw[:])
                wsum.append(ws)

            # Dummy matmul: syncs PE against the ab DMA lane so the real
            # shift matmuls carry only the DVE (s_ext) wait.
            junk = pwpool.tile([1, 1], F32, name="junk", tag="junk")
            nc.tensor.matmul(
                junk[:], lhsT=ab_sb[:, 0:1], rhs=ab_sb[:, 0:1],
                start=True, stop=True,
            )

            # s_ext[b][:, 1+t] = s for tile t; cols 0 and TPB+1 stay zero so
            # the neighbor-tile matmul can read past either end. Reduce per
            # load-quarter so downstream work starts as chunks land.
            s_ext = []
            for b in range(BPC):
                xv = xts[b][:].rearrange("p (t c) -> p t c", c=CIN)
                se = cpool.tile([128, TPB + 2], F32, name=f"s_ext{b}")
                nc.vector.memset(se[:], 0.0)
                for q in range(NQ):
                    nc.vector.reduce_sum(
                        out=se[:, 1 + q * qt:1 + (q + 1) * qt],
                        in_=xv[:, q * qt:(q + 1) * qt],
                        axis=mybir.AxisListType.X,
                    )
                s_ext.append(se)

            # Center tap first: it depends only on the reduce, not on the
            # shift matmuls, so output DMA starts earliest.
            for n, (dx, dy) in sorted(enumerate(TAPS), key=lambda e: e[1] != (0, 0)):
                d = 96 * dx + dy
                for b in range(BPC):
                    se = s_ext[b]
                    if d == 0:
                        ssh, off = se, 1
                    else:
                        ps = pspool.tile([128, TPB], F32, name=f"ps{n}_{b}", tag="ps")
                        nc.tensor.matmul(
                            ps[:], lhsT=ab_sb[:, n * 128:(n + 1) * 128],
                            rhs=se[:, 1:TPB + 1], start=True, stop=False,
                        )
                        rhs2 = se[:, 2:TPB + 2] if d > 0 else se[:, 0:TPB]
                        nc.tensor.matmul(
                            ps[:], lhsT=ab_sb[:, (NTAP + n) * 128:(NTAP + n + 1) * 128],
                            rhs=rhs2, start=False, stop=True,
                        )
                        st = spool.tile([128, TPB], F32, name=f"ssh{n}_{b}", tag="ssh")
                        if dy != 0:
                            mc = 0 if dy == -1 else 1
                            nc.vector.tensor_mul(
                                st[:], ps[:], mk_sb[:, mc * TPB:(mc + 1) * TPB]
                            )
                        else:
                            nc.vector.tensor_copy(st[:], ps[:])
                        ssh, off = st, 0

                    for h in range(2):
                        t0, t1 = h * (TPB // 2), (h + 1) * (TPB // 2)
                        slab = opool.tile(
                            [128, (TPB // 2) * COUT], F32,
                            name=f"slab{n}_{b}_{h}", tag="slab",
                        )
                        for t in range(t0, t1):
                            dst = slab[:, (t - t0) * COUT:(t - t0 + 1) * COUT]
                            sc = ssh[:, off + t:off + t + 1]
                            if t % 3 == 2:
                                nc.scalar.mul(dst, wsum[n][:], sc)
                            else:
                                nc.vector.tensor_scalar_mul(dst, wsum[n][:], sc)
                        nc.sync.dma_start(
                            out=y[n, b][:, t0 * COUT:t1 * COUT], in_=slab[:]
                        )
    nc.finalize()
    return nc


_CACHE = {}


def _get_nc():
    if "nc" not in _CACHE:
        _CACHE["nc"] = _build_nc()
        _CACHE["consts"] = _build_consts()
    return _CACHE["nc"], _CACHE["consts"]


def _run(x_full, w_full, **kwargs):
    nc, (shift_ab, masks) = _get_nc()
    wc = np.concatenate(
        [np.ones((1, 128, COUT), np.float32), w_full.reshape(NTAP, 128, COUT)]
    )
    xr = x_full.reshape(NCORES, BPC, SP, CIN)
    in_maps = [
        {
            "x": np.ascontiguousarray(xr[c]),
            "wc": wc,
            "ab": shift_ab,
            "mk": masks,
        }
        for c in range(NCORES)
    ]
    return run_bass_kernel_spmd(nc, in_maps, core_ids=list(range(NCORES)), **kwargs)


def _unshard(results):
    """Per-core y is [9, BPC, 128(p), 72(t)*128(o)]; spatial index is
    f = 128*t + p, so permute (p,t)->(t,p) while gathering."""
    out = np.empty((3, 3, B, H, W_, COUT), np.float32)
    ov = out.reshape(NTAP, B, TPB, 128, COUT)
    for c, r in enumerate(results):
        yc = r["y"].reshape(NTAP, BPC, 128, TPB, COUT)
        ov[:, BPC * c:BPC * (c + 1)] = yc.transpose(0, 1, 3, 2, 4)
    return out


def kernel(**inputs):
    x_full = np.ascontiguousarray(np.asarray(inputs["inputs"], dtype=np.float32))
    w_full = np.ascontiguousarray(np.asarray(inputs["W"], dtype=np.float32))
    res = _run(x_full, w_full)
    return _unshard(res.results)



# revision 3
# speedup vs baseline: 2.1145x; 2.1145x over previous
"""Trainium2 Bass kernel for nn_ConvBundle_48146583388363.

Math: out[x,y,b,i,j,o] = s[b, i+x-1, j+y-1] * wsum[x,y,o]
  where s = inputs.sum(channel) (zero-padded at borders) and
  wsum = W.sum(axis=2).

Sharding: data-parallel over batch B=16 across 8 cores (2 batches/core).
W and the small structural constants are replicated.

Per-core pipeline (output-bandwidth bound; ~43 MB of bf16 writes/core):
  1. x arrives host-transposed as [cin=64(p), 9216(f)] bf16 -> dense loads.
  2. ones[64,128]^T @ x_chunk matmuls channel-reduce AND broadcast s to all
     128 partitions in one PE op; ACT drains PSUM into a zero-padded bf16
     s vector [128, 97+9216+97] (s replicated per partition).
  3. Each tap shift is a pure AP offset into the padded s. A quarter-slab
     [128(cout), 2304(f)] is ONE dense DVE tensor_scalar_mul with the
     per-partition scalar wsumT[o] = W[tap].sum(cin)[o]; column-border
     masks are strided memsets of 24 columns.
  4. Slabs DMA out as [cout(p), f] bf16; host unshard transposes to
     [..., f, cout] and upcasts to f32 (rel-err of bf16 ~2e-3 << 2e-2).
"""

import numpy as np
import ml_dtypes

import concourse.bacc as bacc
import concourse.bass as bass
import concourse.mybir as mybir
from concourse import tile
from concourse.bass_utils import run_bass_kernel_spmd

F32 = mybir.dt.float32
BF16 = mybir.dt.bfloat16

NCORES = 8
B, H, W_, CIN = 16, 96, 96, 64
COUT = 128
BPC = B // NCORES          # batches per core = 2
SP = H * W_                # 9216 spatial positions per batch
NTAP = 9
TAPS = [(x - 1, y - 1) for x in range(3) for y in range(3)]  # tap n = 3x+y
PAD = 97                   # max |96*dx + dy|
CH = 512                   # s-broadcast matmul chunk = one PSUM bank
NCH = SP // CH             # 18
NQ = 4                     # output slab quarters
QF = SP // NQ              # 2304


def _build_nc():
    # Bacc (not raw Bass): its finalize() runs move_matmul_waits_to_ldweights
    # + generate_event_semaphores, which split multi-waits to satisfy the
    # 1-sync-wait-per-instruction hardware constraint.
    nc = bacc.Bacc(None, target_bir_lowering=False)
    x = nc.dram_tensor("x", [BPC, CIN, SP], BF16, kind="ExternalInput")
    w = nc.dram_tensor("w", [NTAP, 128, COUT], F32, kind="ExternalInput")
    on = nc.dram_tensor("on", [CIN, 128], BF16, kind="ExternalInput")
    onf = nc.dram_tensor("onf", [128, 1], F32, kind="ExternalInput")
    # y stored (o, f) per (tap, batch): cout-major so each partition's 9216
    # bf16 values are one contiguous 18.4KB DRAM run; host transposes back.
    y = nc.dram_tensor("y", [NTAP, BPC, COUT, SP], BF16, kind="ExternalOutput")

    with tile.TileContext(nc) as tc:
        with (
            tc.tile_pool(name="const", bufs=1) as cpool,
            tc.tile_pool(name="xin", bufs=2) as xpool,
            tc.tile_pool(name="psum_w", bufs=1, space="PSUM") as pwpool,
            tc.tile_pool(name="psum_s", bufs=6, space="PSUM") as pspool,
            tc.tile_pool(name="out", bufs=12) as opool,
        ):
            # Batch 0 quarters split across both HWDGE rings (it gates the
            # first slabs); consts on the sync ring; batch 1 on scalar.
            xts = [
                xpool.tile([CIN, SP], BF16, name=f"xt{b}", tag="xt")
                for b in range(BPC)
            ]
            for q in range(NQ):
                eng = nc.scalar if q % 2 == 0 else nc.sync
                eng.dma_start(
                    out=xts[0][:, q * QF:(q + 1) * QF],
                    in_=x[0][:, q * QF:(q + 1) * QF],
                )
            w_sb = cpool.tile([128, NTAP * COUT], F32, name="w_sb")
            nc.sync.dma_start(out=w_sb[:], in_=w.rearrange("n k m -> k n m"))
            on_sb = cpool.tile([CIN, 128], BF16, name="on_sb")
            nc.sync.dma_start(out=on_sb[:], in_=on[:, :])
            onf_sb = cpool.tile([128, 1], F32, name="onf_sb")
            nc.sync.dma_start(out=onf_sb[:], in_=onf[:, :])
            for q in range(NQ):
                nc.scalar.dma_start(
                    out=xts[1][:, q * QF:(q + 1) * QF],
                    in_=x[1][:, q * QF:(q + 1) * QF],
                )

            # Dummy matmuls: pre-sync PE against the const DMA lanes so real
            # matmuls carry only their data-operand wait.
            junk = pwpool.tile([1, 1], F32, name="junk", tag="junk")
            nc.tensor.matmul(
                junk[:], lhsT=on_sb[:, 0:1], rhs=on_sb[:, 0:1],
                start=True, stop=True,
            )
            junk2 = pwpool.tile([1, 1], F32, name="junk2", tag="junk")
            nc.tensor.matmul(
                junk2[:], lhsT=onf_sb[:], rhs=onf_sb[:], start=True, stop=True,
            )

            # wsumT[:, n] = colsum of W[n] with cout on partitions, f32.
            wsumT = cpool.tile([128, NTAP], F32, name="wsumT")
            for n in range(NTAP):
                pw = pwpool.tile([128, 1], F32, name=f"pw{n}", tag="pw")
                nc.tensor.matmul(
                    pw[:], lhsT=w_sb[:, n * COUT:(n + 1) * COUT],
                    rhs=onf_sb[:], start=True, stop=True,
                )
                nc.scalar.copy(wsumT[:, n:n + 1], pw[:])

            # s replicated across all 128 partitions, zero-padded both sides.
            svar = []
            for b in range(BPC):
                sv = cpool.tile([128, PAD + SP + PAD], BF16, name=f"sv{b}")
                nc.vector.memset(sv[:, 0:PAD], 0.0)
                nc.vector.memset(sv[:, PAD + SP:], 0.0)
                for k in range(NCH):
                    ps = pspool.tile([128, CH], F32, name=f"ps{b}_{k}", tag="ps")
                    nc.tensor.matmul(
                        ps[:], lhsT=on_sb[:],
                        rhs=xts[b][:, k * CH:(k + 1) * CH],
                        start=True, stop=True,
                    )
                    nc.scalar.copy(sv[:, PAD + k * CH:PAD + (k + 1) * CH], ps[:])
                svar.append(sv)

            # Center tap first within each quarter: earliest output DMA.
            order = sorted(range(NTAP), key=lambda n: TAPS[n] != (0, 0))
            for b in range(BPC):
                for q in range(NQ):
                    for n in order:
                        dx, dy = TAPS[n]
                        d = 96 * dx + dy
                        slab = opool.tile(
                            [128, QF], BF16, name=f"sl{n}_{b}_{q}", tag="slab"
                        )
                        nc.vector.tensor_scalar_mul(
                            slab[:],
                            svar[b][:, PAD + d + q * QF:PAD + d + (q + 1) * QF],
                            wsumT[:, n:n + 1],
                        )
                        if dy != 0:
                            j = 0 if dy == -1 else 95
                            nc.vector.memset(
                                slab[:].rearrange("p (i j) -> p i j", j=96)
                                [:, :, j:j + 1],
                                0.0,
                            )
                        nc.sync.dma_start(
                            out=y[n, b][:, q * QF:(q + 1) * QF], in_=slab[:]
                        )
    nc.finalize()
    return nc


_CACHE = {}


def _get_nc():
    if "nc" not in _CACHE:
        _CACHE["nc"] = _build_nc()
    return _CACHE["nc"]


def _run(x_full, w_full, **kwargs):
    nc = _get_nc()
    xr = x_full.reshape(NCORES, BPC, SP, CIN)
    ones = np.ones((CIN, 128), ml_dtypes.bfloat16)
    onesf = np.ones((128, 1), np.float32)
    wr = np.ascontiguousarray(w_full.reshape(NTAP, 128, COUT))
    in_maps = [
        {
            # host-side transpose: [sp, cin] -> [cin, sp], cast to bf16
            "x": xr[c].transpose(0, 2, 1).astype(ml_dtypes.bfloat16),
            "w": wr,
            "on": ones,
            "onf": onesf,
        }
        for c in range(NCORES)
    ]
    return run_bass_kernel_spmd(nc, in_maps, core_ids=list(range(NCORES)), **kwargs)


def _unshard(results):
    """Per-core y is [9, BPC, 128(o), 9216(f)] bf16; transpose to
    [..., f, o] while gathering and upcast to f32."""
    out = np.empty((3, 3, B, H, W_, COUT), np.float32)
    ov = out.reshape(NTAP, B, SP, COUT)
    for c, r in enumerate(results):
        yc = np.asarray(r["y"]).astype(np.float32)
        ov[:, BPC * c:BPC * (c + 1)] = yc.transpose(0, 1, 3, 2)
    return out


def kernel(**inputs):
    x_full = np.ascontiguousarray(np.asarray(inputs["inputs"], dtype=np.float32))
    w_full = np.ascontiguousarray(np.asarray(inputs["W"], dtype=np.float32))
    res = _run(x_full, w_full)
    return _unshard(res.results)


# revision 6
# speedup vs baseline: 2.2172x; 1.0486x over previous
"""Trainium2 Bass kernel for nn_ConvBundle_48146583388363.

Math: out[x,y,b,i,j,o] = s[b, i+x-1, j+y-1] * wsum[x,y,o]
  where s = inputs.sum(channel) (zero-padded at borders) and
  wsum = W.sum(axis=2).

Sharding: data-parallel over batch B=16 across 8 cores (2 batches/core).
W and the small structural constants are replicated.

Per-core pipeline (output-bandwidth bound; ~43 MB of bf16 writes/core):
  1. x arrives host-transposed as [cin=64(p), 9216(f)] bf16 -> dense loads.
  2. ones[64,128]^T @ x_chunk matmuls channel-reduce AND broadcast s to all
     128 partitions in one PE op; ACT drains PSUM into a zero-padded bf16
     s vector [128, 97+9216+97] (s replicated per partition).
  3. Each tap shift is a pure AP offset into the padded s. A quarter-slab
     [128(cout), 2304(f)] is ONE dense DVE tensor_scalar_mul with the
     per-partition scalar wsumT[o] = W[tap].sum(cin)[o]; column-border
     masks are strided memsets of 24 columns.
  4. Slabs DMA out as [cout(p), f] bf16; host unshard transposes to
     [..., f, cout] and upcasts to f32 (rel-err of bf16 ~2e-3 << 2e-2).
"""

import numpy as np
import ml_dtypes

import concourse.bacc as bacc
import concourse.bass as bass
import concourse.mybir as mybir
from concourse import tile
from concourse.bass_utils import run_bass_kernel_spmd

F32 = mybir.dt.float32
BF16 = mybir.dt.bfloat16

NCORES = 8
B, H, W_, CIN = 16, 96, 96, 64
COUT = 128
BPC = B // NCORES          # batches per core = 2
SP = H * W_                # 9216 spatial positions per batch
NTAP = 9
TAPS = [(x - 1, y - 1) for x in range(3) for y in range(3)]  # tap n = 3x+y
PAD = 97                   # max |96*dx + dy|
CH = 512                   # s-broadcast matmul chunk = one PSUM bank
NCH = SP // CH             # 18
NQ = 4                     # output slab quarters
QF = SP // NQ              # 2304


def _build_nc():
    # Bacc (not raw Bass): its finalize() runs move_matmul_waits_to_ldweights
    # + generate_event_semaphores, which split multi-waits to satisfy the
    # 1-sync-wait-per-instruction hardware constraint.
    nc = bacc.Bacc(None, target_bir_lowering=False)
    x = nc.dram_tensor("x", [BPC, CIN, SP], BF16, kind="ExternalInput")
    # w host-pretransposed to [cin, tap*cout]: one dense linear DMA
    w = nc.dram_tensor("w", [128, NTAP * COUT], F32, kind="ExternalInput")
    on = nc.dram_tensor("on", [CIN, 128], BF16, kind="ExternalInput")
    onf = nc.dram_tensor("onf", [128, 1], F32, kind="ExternalInput")
    # y stored (o, f) per (tap, batch): cout-major so each partition's 9216
    # bf16 values are one contiguous 18.4KB DRAM run; host transposes back.
    y = nc.dram_tensor("y", [NTAP, BPC, COUT, SP], BF16, kind="ExternalOutput")

    with tile.TileContext(nc) as tc:
        with (
            tc.tile_pool(name="const", bufs=1) as cpool,
            tc.tile_pool(name="xin", bufs=2) as xpool,
            tc.tile_pool(name="psum_w", bufs=1, space="PSUM") as pwpool,
            tc.tile_pool(name="psum_s", bufs=6, space="PSUM") as pspool,
            tc.tile_pool(name="out", bufs=12) as opool,
        ):
            # Batch 0 quarters split across both HWDGE rings (it gates the
            # first slabs); consts on the sync ring; batch 1 on scalar.
            xts = [
                xpool.tile([CIN, SP], BF16, name=f"xt{b}", tag="xt")
                for b in range(BPC)
            ]
            # Consts first on the sync ring (tiny; gate the PE pipeline),
            # then the dense w, then batch-0 spill quarters.
            on_sb = cpool.tile([CIN, 128], BF16, name="on_sb")
            nc.sync.dma_start(out=on_sb[:], in_=on[:, :])
            onf_sb = cpool.tile([128, 1], F32, name="onf_sb")
            nc.sync.dma_start(out=onf_sb[:], in_=onf[:, :])
            w_sb = cpool.tile([128, NTAP * COUT], F32, name="w_sb")
            nc.sync.dma_start(out=w_sb[:], in_=w[:, :])
            for q in range(NQ):
                eng = nc.scalar if q % 2 == 0 else nc.sync
                eng.dma_start(
                    out=xts[0][:, q * QF:(q + 1) * QF],
                    in_=x[0][:, q * QF:(q + 1) * QF],
                )
            for q in range(NQ):
                nc.scalar.dma_start(
                    out=xts[1][:, q * QF:(q + 1) * QF],
                    in_=x[1][:, q * QF:(q + 1) * QF],
                )

            # Dummy matmuls: pre-sync PE against the const DMA lanes so real
            # matmuls carry only their data-operand wait.
            junk = pwpool.tile([1, 1], F32, name="junk", tag="junk")
            nc.tensor.matmul(
                junk[:], lhsT=on_sb[:, 0:1], rhs=on_sb[:, 0:1],
                start=True, stop=True,
            )
            junk2 = pwpool.tile([1, 1], F32, name="junk2", tag="junk")
            nc.tensor.matmul(
                junk2[:], lhsT=onf_sb[:], rhs=onf_sb[:], start=True, stop=True,
            )

            # wsumT[:, n] = colsum of W[n] with cout on partitions, f32.
            wsumT = cpool.tile([128, NTAP], F32, name="wsumT")
            for n in range(NTAP):
                pw = pwpool.tile([128, 1], F32, name=f"pw{n}", tag="pw")
                nc.tensor.matmul(
                    pw[:], lhsT=w_sb[:, n * COUT:(n + 1) * COUT],
                    rhs=onf_sb[:], start=True, stop=True,
                )
                nc.scalar.copy(wsumT[:, n:n + 1], pw[:])

            # s replicated across all 128 partitions, zero-padded both sides.
            svar = []
            for b in range(BPC):
                sv = cpool.tile([128, PAD + SP + PAD], BF16, name=f"sv{b}")
                nc.vector.memset(sv[:, 0:PAD], 0.0)
                nc.vector.memset(sv[:, PAD + SP:], 0.0)
                for k in range(NCH):
                    ps = pspool.tile([128, CH], F32, name=f"ps{b}_{k}", tag="ps")
                    nc.tensor.matmul(
                        ps[:], lhsT=on_sb[:],
                        rhs=xts[b][:, k * CH:(k + 1) * CH],
                        start=True, stop=True,
                    )
                    nc.scalar.copy(sv[:, PAD + k * CH:PAD + (k + 1) * CH], ps[:])
                svar.append(sv)

            # Center tap first within each quarter: earliest output DMA.
            order = sorted(range(NTAP), key=lambda n: TAPS[n] != (0, 0))
            for b in range(BPC):
                for q in range(NQ):
                    for n in order:
                        dx, dy = TAPS[n]
                        d = 96 * dx + dy
                        slab = opool.tile(
                            [128, QF], BF16, name=f"sl{n}_{b}_{q}", tag="slab"
                        )
                        nc.vector.tensor_scalar_mul(
                            slab[:],
                            svar[b][:, PAD + d + q * QF:PAD + d + (q + 1) * QF],
                            wsumT[:, n:n + 1],
                        )
                        if dy != 0:
                            j = 0 if dy == -1 else 95
                            nc.vector.memset(
                                slab[:].rearrange("p (i j) -> p i j", j=96)
                                [:, :, j:j + 1],
                                0.0,
                            )
                        nc.sync.dma_start(
                            out=y[n, b][:, q * QF:(q + 1) * QF], in_=slab[:]
                        )
    nc.finalize()
    return nc


_CACHE = {}


def _get_nc():
    if "nc" not in _CACHE:
        _CACHE["nc"] = _build_nc()
    return _CACHE["nc"]


def _run(x_full, w_full, **kwargs):
    nc = _get_nc()
    xr = x_full.reshape(NCORES, BPC, SP, CIN)
    ones = np.ones((CIN, 128), ml_dtypes.bfloat16)
    onesf = np.ones((128, 1), np.float32)
    wr = np.ascontiguousarray(
        w_full.reshape(NTAP, 128, COUT).transpose(1, 0, 2).reshape(128, NTAP * COUT)
    )
    in_maps = [
        {
            # host-side transpose: [sp, cin] -> [cin, sp], cast to bf16
            "x": xr[c].transpose(0, 2, 1).astype(ml_dtypes.bfloat16),
            "w": wr,
            "on": ones,
            "onf": onesf,
        }
        for c in range(NCORES)
    ]
    return run_bass_kernel_spmd(nc, in_maps, core_ids=list(range(NCORES)), **kwargs)


def _unshard(results):
    """Per-core y is [9, BPC, 128(o), 9216(f)] bf16; transpose to
    [..., f, o] while gathering and upcast to f32."""
    out = np.empty((3, 3, B, H, W_, COUT), np.float32)
    ov = out.reshape(NTAP, B, SP, COUT)
    for c, r in enumerate(results):
        yc = np.asarray(r["y"]).astype(np.float32)
        ov[:, BPC * c:BPC * (c + 1)] = yc.transpose(0, 1, 3, 2)
    return out


def kernel(**inputs):
    x_full = np.ascontiguousarray(np.asarray(inputs["inputs"], dtype=np.float32))
    w_full = np.ascontiguousarray(np.asarray(inputs["W"], dtype=np.float32))
    res = _run(x_full, w_full)
    return _unshard(res.results)


# revision 10
# speedup vs baseline: 2.2731x; 1.0252x over previous
"""Trainium2 Bass kernel for nn_ConvBundle_48146583388363.

Math: out[x,y,b,i,j,o] = s[b, i+x-1, j+y-1] * wsum[x,y,o]
  where s = inputs.sum(channel) (zero-padded at borders) and
  wsum = W.sum(axis=2).

Sharding: data-parallel over batch B=16 across 8 cores (2 batches/core).
W and the small structural constants are replicated.

Per-core pipeline (output-bandwidth bound; ~43 MB of bf16 writes/core):
  1. x arrives host-transposed as [cin=64(p), 9216(f)] bf16 -> dense loads.
  2. ones[64,128]^T @ x_chunk matmuls channel-reduce AND broadcast s to all
     128 partitions in one PE op; ACT drains PSUM into a zero-padded bf16
     s vector [128, 97+9216+97] (s replicated per partition).
  3. Each tap shift is a pure AP offset into the padded s. A quarter-slab
     [128(cout), 2304(f)] is ONE dense DVE tensor_scalar_mul with the
     per-partition scalar wsumT[o] = W[tap].sum(cin)[o]; column-border
     masks are strided memsets of 24 columns.
  4. Slabs DMA out as [cout(p), f] bf16; host unshard transposes to
     [..., f, cout] and upcasts to f32 (rel-err of bf16 ~2e-3 << 2e-2).
"""

import numpy as np
import ml_dtypes

import concourse.bacc as bacc
import concourse.bass as bass
import concourse.mybir as mybir
from concourse import tile
from concourse.bass_utils import run_bass_kernel_spmd

F32 = mybir.dt.float32
BF16 = mybir.dt.bfloat16

NCORES = 8
B, H, W_, CIN = 16, 96, 96, 64
COUT = 128
BPC = B // NCORES          # batches per core = 2
SP = H * W_                # 9216 spatial positions per batch
NTAP = 9
TAPS = [(x - 1, y - 1) for x in range(3) for y in range(3)]  # tap n = 3x+y
PAD = 97                   # max |96*dx + dy|
CH = 512                   # s-broadcast matmul chunk = one PSUM bank
NCH = SP // CH             # 18
NQ = 4                     # output slab quarters
QF = SP // NQ              # 2304


def _build_nc():
    # Bacc (not raw Bass): its finalize() runs move_matmul_waits_to_ldweights
    # + generate_event_semaphores, which split multi-waits to satisfy the
    # 1-sync-wait-per-instruction hardware constraint.
    nc = bacc.Bacc(None, target_bir_lowering=False)
    # x viewed as [128, 4608]: partition 2c+h holds channel c, f-half h
    # (full 128-partition DMA spray; a [64, SP] layout runs at half rate).
    x = nc.dram_tensor("x", [BPC, 128, SP // 2], BF16, kind="ExternalInput")
    # w host-pretransposed to [cin, tap*cout]: one dense linear DMA. bf16 so
    # the wsumT matmuls are single-pass (f32 PE matmul = slow double-pass).
    w = nc.dram_tensor("w", [128, NTAP * COUT], BF16, kind="ExternalInput")
    # on[k, 128h:128h+128] = (k%2==h): lhsT masks that channel-sum the
    # even/odd partitions (= f-half h) of the x tile.
    on = nc.dram_tensor("on", [128, 256], BF16, kind="ExternalInput")
    onf = nc.dram_tensor("onf", [128, 1], BF16, kind="ExternalInput")
    # y stored (o, f) per (tap, batch): cout-major so each partition's 9216
    # bf16 values are one contiguous 18.4KB DRAM run; host transposes back.
    y = nc.dram_tensor("y", [NTAP, BPC, COUT, SP], BF16, kind="ExternalOutput")

    with tile.TileContext(nc) as tc:
        with (
            tc.tile_pool(name="const", bufs=1) as cpool,
            tc.tile_pool(name="xin", bufs=2) as xpool,
            tc.tile_pool(name="psum_w", bufs=1, space="PSUM") as pwpool,
            tc.tile_pool(name="psum_s", bufs=6, space="PSUM") as pspool,
            tc.tile_pool(name="out", bufs=12) as opool,
        ):
            # Batch 0 column-halves split across both HWDGE rings (it gates
            # the first slabs); consts on the sync ring; batch 1 on scalar.
            HC = SP // 4  # 2304 columns per half of the [128, 4608] tile
            xts = [
                xpool.tile([128, SP // 2], BF16, name=f"xt{b}", tag="xt")
                for b in range(BPC)
            ]
            # Consts first on the sync ring (tiny; gate the PE pipeline).
            on_sb = cpool.tile([128, 256], BF16, name="on_sb")
            nc.sync.dma_start(out=on_sb[:], in_=on[:, :])
            onf_sb = cpool.tile([128, 1], BF16, name="onf_sb")
            nc.sync.dma_start(out=onf_sb[:], in_=onf[:, :])
            w_sb = cpool.tile([128, NTAP * COUT], BF16, name="w_sb")
            nc.sync.dma_start(out=w_sb[:], in_=w[:, :])
            nc.scalar.dma_start(out=xts[0][:, 0:HC], in_=x[0][:, 0:HC])
            nc.sync.dma_start(out=xts[0][:, HC:2 * HC], in_=x[0][:, HC:2 * HC])
            nc.scalar.dma_start(out=xts[1][:, 0:HC], in_=x[1][:, 0:HC])
            nc.scalar.dma_start(out=xts[1][:, HC:2 * HC], in_=x[1][:, HC:2 * HC])

            # Dummy matmuls: pre-sync PE against the const DMA lanes so real
            # matmuls carry only their data-operand wait.
            junk = pwpool.tile([1, 1], F32, name="junk", tag="junk")
            nc.tensor.matmul(
                junk[:], lhsT=on_sb[:, 0:1], rhs=on_sb[:, 0:1],
                start=True, stop=True,
            )
            junk2 = pwpool.tile([1, 1], F32, name="junk2", tag="junk")
            nc.tensor.matmul(
                junk2[:], lhsT=onf_sb[:], rhs=onf_sb[:], start=True, stop=True,
            )

            # wsumT[:, n] = colsum of W[n] with cout on partitions: 9 single-
            # pass matmuls into one PSUM tile, one ACT copy out (f32 scalar).
            wsumT = cpool.tile([128, NTAP], F32, name="wsumT")
            pwall = pwpool.tile([128, NTAP], F32, name="pwall", tag="pw")
            for n in range(NTAP):
                nc.tensor.matmul(
                    pwall[:, n:n + 1], lhsT=w_sb[:, n * COUT:(n + 1) * COUT],
                    rhs=onf_sb[:], start=True, stop=True,
                )
            nc.scalar.copy(wsumT[:], pwall[:])

            # s replicated across all 128 partitions, zero-padded both sides.
            # Chunk kk covers f in [512kk, 512kk+512): h = kk//9 picks the
            # even/odd lhsT mask, j = kk%9 the column chunk.
            svar = []
            for b in range(BPC):
                sv = cpool.tile([128, PAD + SP + PAD], BF16, name=f"sv{b}")
                nc.vector.memset(sv[:, 0:PAD], 0.0)
                nc.vector.memset(sv[:, PAD + SP:], 0.0)
                for kk in range(NCH):
                    h, j = kk // 9, kk % 9
                    ps = pspool.tile([128, CH], F32, name=f"ps{b}_{kk}", tag="ps")
                    nc.tensor.matmul(
                        ps[:], lhsT=on_sb[:, 128 * h:128 * (h + 1)],
                        rhs=xts[b][:, j * CH:(j + 1) * CH],
                        start=True, stop=True,
                    )
                    nc.scalar.copy(sv[:, PAD + kk * CH:PAD + (kk + 1) * CH], ps[:])
                svar.append(sv)

            # Center tap first within each quarter: earliest output DMA.
            order = sorted(range(NTAP), key=lambda n: TAPS[n] != (0, 0))
            for b in range(BPC):
                for q in range(NQ):
                    for n in order:
                        dx, dy = TAPS[n]
                        d = 96 * dx + dy
                        slab = opool.tile(
                            [128, QF], BF16, name=f"sl{n}_{b}_{q}", tag="slab"
                        )
                        nc.vector.tensor_scalar_mul(
                            slab[:],
                            svar[b][:, PAD + d + q * QF:PAD + d + (q + 1) * QF],
                            wsumT[:, n:n + 1],
                        )
                        if dy != 0:
                            j = 0 if dy == -1 else 95
                            nc.vector.memset(
                                slab[:].rearrange("p (i j) -> p i j", j=96)
                                [:, :, j:j + 1],
                                0.0,
                            )
                        nc.sync.dma_start(
                            out=y[n, b][:, q * QF:(q + 1) * QF], in_=slab[:]
                        )
    nc.finalize()
    return nc


_CACHE = {}


def _get_nc():
    if "nc" not in _CACHE:
        _CACHE["nc"] = _build_nc()
    return _CACHE["nc"]


def _run(x_full, w_full, **kwargs):
    nc = _get_nc()
    xr = x_full.reshape(NCORES, BPC, SP, CIN)
    ones = np.zeros((128, 256), ml_dtypes.bfloat16)
    ones[0::2, 0:128] = 1
    ones[1::2, 128:256] = 1
    onesf = np.ones((128, 1), ml_dtypes.bfloat16)
    wr = np.ascontiguousarray(
        w_full.reshape(NTAP, 128, COUT).transpose(1, 0, 2).reshape(128, NTAP * COUT)
    ).astype(ml_dtypes.bfloat16)
    in_maps = [
        {
            # host-side transpose: [sp, cin] -> [cin, sp], cast to bf16,
            # viewed [128, 4608] (partition 2c+h = channel c, f-half h)
            "x": xr[c].transpose(0, 2, 1).astype(ml_dtypes.bfloat16)
                 .reshape(BPC, 128, SP // 2),
            "w": wr,
            "on": ones,
            "onf": onesf,
        }
        for c in range(NCORES)
    ]
    return run_bass_kernel_spmd(nc, in_maps, core_ids=list(range(NCORES)), **kwargs)


def _unshard(results):
    """Per-core y is [9, BPC, 128(o), 9216(f)] bf16; transpose to
    [..., f, o] while gathering and upcast to f32."""
    out = np.empty((3, 3, B, H, W_, COUT), np.float32)
    ov = out.reshape(NTAP, B, SP, COUT)
    for c, r in enumerate(results):
        yc = np.asarray(r["y"]).astype(np.float32)
        ov[:, BPC * c:BPC * (c + 1)] = yc.transpose(0, 1, 3, 2)
    return out


def kernel(**inputs):
    x_full = np.ascontiguousarray(np.asarray(inputs["inputs"], dtype=np.float32))
    w_full = np.ascontiguousarray(np.asarray(inputs["W"], dtype=np.float32))
    res = _run(x_full, w_full)
    return _unshard(res.results)
